# revision 1
# baseline (speedup 1.0000x reference)
"""Trainium2 Bass kernel for complex-valued sparse attention.

Model (B=2, L=2048, D=1024, H=16 heads, DH=64, G=64 global tokens):
  Q/K/V complex projections, real-part scores softmax(Re(Q K^H)) with key
  mask, plus a learned global-token branch, then complex output projection.

Sharding: 8 cores = 2 (batch) x 4 (head groups of 4 heads).  Each core
computes its batch element restricted to its 4 heads end-to-end (column
shards of Wq/Wk/Wv, row shards of Wo) and returns a partial [D, L] output
(transposed); the host sums the 4 head-group partials per batch element.

Key ideas:
  - SPARSITY: masked keys contribute exp(-inf)=0, so the host gathers the
    unmasked key positions (~L/2) and the kernel only projects/attends
    over LK = padded gathered keys.  The Bass program is built after the
    mask is known, so LK is a compile-time constant per run; pad columns
    are zeros with a -60 additive bias (exp -> ~1e-26).
  - Seq-transposed activations: QcT/KcT are [c=128, seq] per head where
    c = (64 real | 64 imag), so score matmuls contract all 128 partitions
    at once: S^T[m,l] = sum_c KcT[c,m] QcT[c,l].  Scores are built
    transposed (keys on partitions) so P@V needs no transpose:
    out^T[c,l] = sum_m Vc[m,c] P^T[m,l].
  - All projections run at M=128 by pairing heads in the stationary
    operand; partition-shifting PSUM->SBUF copies repack into per-head
    layout.
  - Softmax skips max-subtraction (scores are O(1) here: weights are
    ~N(0, 0.02^2)).  The denominator alternates engines per head: even
    heads reduce on the PE (broadcast ones-matmul), odd heads on DVE
    chunk adds + GPSIMD partition all-reduce.
  - fp32 data feeds the PE as float32r (full rate at free dim >= 256).
"""

import numpy as np

import concourse.mybir as mybir
import concourse.tile as tile
from concourse import bacc, bass_isa
from concourse.bass_utils import run_bass_kernel_spmd

B, L, D, H, G = 2, 2048, 1024, 16, 64
DH = D // H            # 64 dims per head
HPC = 4                # heads per core
NPAIR = HPC // 2       # head pairs per core
CPH = HPC * DH         # 256 projection columns per core
NCORES = 8
SCALE = DH ** -0.5     # 0.125
LB = 512               # l-block width in attention / output phases
NLB = L // LB          # 4
PB = 256               # seq-block width in projection phase
NPB = L // PB          # 8
NDC = D // 128         # 8 contraction chunks of 128
NNT = D // 128         # 8 output-column tiles
MASK_BIAS = -60.0      # additive pre-softmax bias for masked/pad keys

F32 = mybir.dt.float32
F32R = mybir.dt.float32r
EXP = mybir.ActivationFunctionType.Exp


def _r(ap):
    """Bitcast an fp32 AP to float32r (same bytes; PE rounds on read)."""
    return ap.bitcast(F32R)


def _build_bass(NKC, NKB):
    import os

    phases = os.environ.get("BASS_PHASES", "123")
    LKP = NKB * PB      # K/V projection width (>= NKC * 128)
    nc = bacc.Bacc()

    din = lambda name, shape: nc.dram_tensor(
        name, shape, F32, kind="ExternalInput"
    ).ap()
    # inputs arrive pre-blocked: [128, block, Dchunk, PB] so each block DMA
    # is one contiguous run per partition
    # the host permutes the sequence unmasked-keys-first, so the key/value
    # projections reuse the first NKB input blocks of the query stream
    rT = din("rT", [128, NPB, NDC, PB])
    iT = din("iT", [128, NPB, NDC, PB])
    wq_r = din("wq_r", [D, NPAIR, 128])   # [:, p] = [Wq cols h2p | h2p+1]
    wq_i = din("wq_i", [D, NPAIR, 128])
    wk_r = din("wk_r", [D, NPAIR, 128])
    wk_i = din("wk_i", [D, NPAIR, 128])
    wv_r = din("wv_r", [D, CPH])
    wv_i = din("wv_i", [D, CPH])
    wo_r = din("wo_r", [NPAIR, 128, D])   # [p] = Wo rows for head pair p
    wo_i = din("wo_i", [NPAIR, 128, D])
    gkc_d = din("gkc", [HPC, 2 * DH, G])
    gvc_d = din("gvc", [HPC, G, 2 * DH])
    maskb_d = din("maskb", [128, NKC])  # attention chunks only
    out_r = nc.dram_tensor("out_r", [D, L], F32, kind="ExternalOutput").ap()
    out_i = nc.dram_tensor("out_i", [D, L], F32, kind="ExternalOutput").ap()

    with tile.TileContext(nc) as tc:
        with (
            nc.allow_low_precision("float32r tiles feed full-rate matmuls"),
            tc.tile_pool(name="persist", bufs=1) as persist,
            tc.tile_pool(name="pmm", bufs=4, space="PSUM") as pmm,
            tc.tile_pool(name="pacc", bufs=1, space="PSUM") as pacc,
        ):
            QcTs = [
                persist.tile(
                    [128, HPC, LB], F32R, tag=f"qc{t}", name=f"QcT{t}"
                )
                for t in range(NLB)
            ]
            KcT = persist.tile([128, HPC, LKP], F32R, tag="kc")
            Vc = persist.tile([128, 2 * NKB, HPC, 128], F32R, tag="vc")
            maskb = persist.tile([128, NKC], F32, tag="mask")
            gkc = persist.tile([128, HPC, G], F32R, tag="gkc")
            gvc = persist.tile([G, HPC, 128], F32R, tag="gvc")
            ones = persist.tile([128, 128], F32R, tag="ones")

            ones_f32 = persist.tile([128, 128], F32, tag="ones_f32")
            nc.vector.memset(ones_f32, 1.0)
            nc.vector.tensor_copy(out=ones, in_=ones_f32)

            # ---------- Phase 1: Q/K/V projections (all M=128) ----------
            with (
                tc.tile_pool(name="wpool", bufs=1) as wpool,
                tc.tile_pool(name="inpool", bufs=3) as inpool,
            ):
                wsb = {}
                for name, ap in (
                    ("wq_r", wq_r),
                    ("wq_i", wq_i),
                    ("wk_r", wk_r),
                    ("wk_i", wk_i),
                ):
                    wsb[name] = wpool.tile(
                        [128, NDC, NPAIR, 128], F32R, tag=name, name=name
                    )

                def load_w(name, ap):
                    v = _r(ap).rearrange("(c p) j n -> p c j n", p=128)
                    for c in range(NDC):
                        nc.sync.dma_start(
                            out=wsb[name][:, c, :, :], in_=v[:, c, :, :]
                        )

                wv_r_sb = wpool.tile([128, NDC, CPH], F32R, tag="wvr")
                wv_i_sb = wpool.tile([128, NDC, CPH], F32R, tag="wvi")

                def proj_pair(ps, w_sb, src_t, dst, p, coff, sl):
                    """One M=128 head-pair projection + shifted repack."""
                    for c in range(NDC):
                        nc.tensor.matmul(
                            ps,
                            w_sb[:, c, p, :],
                            src_t[:, c, :],
                            start=(c == 0),
                            stop=(c == NDC - 1),
                        )
                    nc.scalar.copy(
                        out=dst[coff : coff + DH, 2 * p, sl], in_=ps[0:DH, :]
                    )
                    nc.scalar.copy(
                        out=dst[coff : coff + DH, 2 * p + 1, sl],
                        in_=ps[DH:128, :],
                    )

                # One pass over the input blocks: Q everywhere, K/V on the
                # first NKB blocks (the permuted gathered keys).  The first
                # input block is queued before the bulk of the weights so
                # the PE starts as early as possible.
                for pb in range(NPB if "1" in phases else 0):
                    sl = slice(pb * PB, (pb + 1) * PB)
                    rt_t = inpool.tile([128, NDC, PB], F32R, tag="rt")
                    it_t = inpool.tile([128, NDC, PB], F32R, tag="it")
                    nc.sync.dma_start(out=rt_t, in_=_r(rT)[:, pb, :, :])
                    nc.sync.dma_start(out=it_t, in_=_r(iT)[:, pb, :, :])
                    if pb == 0:
                        load_w("wq_r", wq_r)
                        load_w("wq_i", wq_i)
                        load_w("wk_r", wk_r)
                        load_w("wk_i", wk_i)
                        nc.sync.dma_start(
                            out=wv_r_sb,
                            in_=_r(wv_r).rearrange("(c p) n -> p c n", p=128),
                        )
                        nc.sync.dma_start(
                            out=wv_i_sb,
                            in_=_r(wv_i).rearrange("(c p) n -> p c n", p=128),
                        )
                    for p in range(NPAIR):
                        for w_sb, src_t, coff in (
                            (wsb["wq_r"], rt_t, 0),
                            (wsb["wq_i"], it_t, DH),
                        ):
                            ps = pmm.tile([128, PB], F32, tag="mm")
                            qsl = slice((pb % 2) * PB, (pb % 2) * PB + PB)
                            proj_pair(
                                ps, w_sb, src_t, QcTs[pb // 2], p, coff, qsl
                            )
                    if pb >= NKB:
                        continue
                    for p in range(NPAIR):
                        for w_sb, src_t, coff in (
                            (wsb["wk_r"], rt_t, 0),
                            (wsb["wk_i"], it_t, DH),
                        ):
                            ps = pmm.tile([128, PB], F32, tag="mm")
                            proj_pair(ps, w_sb, src_t, KcT, p, coff, sl)
                    for ms in range(PB // 128):
                        mc = pb * (PB // 128) + ms
                        msl = slice(ms * 128, (ms + 1) * 128)
                        for src_t, wv_sb, coff in (
                            (rt_t, wv_r_sb, 0),
                            (it_t, wv_i_sb, DH),
                        ):
                            ps = pmm.tile([128, CPH], F32, tag="mm")
                            for c in range(NDC):
                                nc.tensor.matmul(
                                    ps,
                                    src_t[:, c, msl],
                                    wv_sb[:, c, :],
                                    start=(c == 0),
                                    stop=(c == NDC - 1),
                                )
                            nc.vector.tensor_copy(
                                out=Vc[:, mc, :, coff : coff + DH],
                                in_=ps.rearrange("p (j d) -> p j d", d=DH),
                            )

            nc.sync.dma_start(out=maskb, in_=maskb_d)
            nc.sync.dma_start(out=gkc, in_=_r(gkc_d).rearrange("j p g -> p j g"))
            nc.sync.dma_start(out=gvc, in_=_r(gvc_d).rearrange("j p c -> p j c"))

            # ---------- Phases 2+3: attention + output projection ----------
            with (
                tc.tile_pool(name="wopool", bufs=1) as wopool,
                tc.tile_pool(name="ptpool", bufs=8) as ptpool,
                tc.tile_pool(name="pgpool", bufs=2) as pgpool,
                tc.tile_pool(name="accpool", bufs=2) as accpool,
                tc.tile_pool(name="outfpool", bufs=8) as outfpool,
                tc.tile_pool(name="rcpool", bufs=6) as rcpool,
                tc.tile_pool(name="ostage", bufs=4) as ostage,
            ):
                wo_r_sb = wopool.tile([128, NPAIR, D], F32R, tag="wor")
                wo_i_sb = wopool.tile([128, NPAIR, D], F32R, tag="woi")
                nc.sync.dma_start(
                    out=wo_r_sb, in_=_r(wo_r).rearrange("p c n -> c p n")
                )
                nc.sync.dma_start(
                    out=wo_i_sb, in_=_r(wo_i).rearrange("p c n -> c p n")
                )

                for lb in range(NLB if "2" in phases else 0):
                    lsl = slice(lb * LB, (lb + 1) * LB)
                    # head-pair layout accumulators for the Wo contraction
                    outf_r = [
                        outfpool.tile(
                            [128, LB], F32R, tag="outf", name=f"outf_r{lb}_{p}"
                        )
                        for p in range(NPAIR)
                    ]
                    outf_i = [
                        outfpool.tile(
                            [128, LB], F32R, tag="outf", name=f"outf_i{lb}_{p}"
                        )
                        for p in range(NPAIR)
                    ]
                    # Per-head tails (global branch + normalize) are
                    # deferred into the NEXT head's chunk stream so the PE
                    # never stalls waiting for the ACT/DVE tail chain.
                    def tail_a(st):
                        j = st["j"]
                        sg = pacc.tile(
                            [G, LB], F32, tag="g", bufs=2, name=f"sg{lb}_{j}"
                        )
                        nc.tensor.matmul(
                            sg,
                            gkc[:, j, :],
                            QcTs[lb][:, j, :],
                            start=True,
                            stop=True,
                        )
                        pgt = pgpool.tile(
                            [G, LB], F32R, tag="pg", name=f"pg{lb}_{j}"
                        )
                        nc.scalar.activation(
                            out=pgt, in_=sg, func=EXP, bias=0.0, scale=SCALE
                        )
                        st["pgt"] = pgt

                    def tail_b(st):
                        j, pv, csb, pgt = st["j"], st["pv"], st["csb"], st["pgt"]
                        p_idx, s_idx = divmod(j, 2)
                        hsl = slice(s_idx * DH, (s_idx + 1) * DH)
                        gcs = pacc.tile(
                            [128, LB], F32, tag="g", bufs=2, name=f"gcs{lb}_{j}"
                        )
                        nc.tensor.matmul(
                            gcs, ones[0:G, :], pgt, start=True, stop=True
                        )
                        gv = pacc.tile(
                            [128, LB], F32, tag="g", bufs=2, name=f"gv{lb}_{j}"
                        )
                        nc.tensor.matmul(
                            gv, gvc[:, j, :], pgt, start=True, stop=True
                        )
                        # DVE cost scales with free size, not partitions:
                        # normalize full-height in 3 ops, then GPSIMD
                        # scatters the (r|i) halves into the pair tiles.
                        rcb = rcpool.tile(
                            [128, LB], F32, tag="rc", name=f"rcb{lb}_{j}"
                        )
                        nc.vector.reciprocal(rcb, csb)
                        rcgb = rcpool.tile(
                            [128, LB], F32, tag="rc", name=f"rcgb{lb}_{j}"
                        )
                        nc.vector.reciprocal(rcgb, gcs)
                        outn = outfpool.tile(
                            [128, LB], F32, tag="tmp", name=f"outn{lb}_{j}"
                        )
                        tg = outfpool.tile(
                            [128, LB], F32, tag="tmp", name=f"tg{lb}_{j}"
                        )
                        nc.vector.tensor_mul(outn, pv, rcb)
                        nc.vector.tensor_mul(tg, gv, rcgb)
                        nc.vector.tensor_add(outn, outn, tg)
                        nc.gpsimd.tensor_copy(
                            out=outf_r[p_idx][hsl, :], in_=outn[0:DH, :]
                        )
                        nc.gpsimd.tensor_copy(
                            out=outf_i[p_idx][hsl, :], in_=outn[DH:128, :]
                        )

                    pending = None
                    for j in range(HPC):
                        pv = pacc.tile([128, LB], F32, tag="pv", bufs=2)
                        acc = accpool.tile(
                            [128, LB], F32, tag="acc", name=f"acc{lb}_{j}"
                        )
                        pts = []
                        for mc in range(NKC):
                            s_ps = pmm.tile([128, LB], F32, tag="mm")
                            nc.tensor.matmul(
                                s_ps,
                                KcT[:, j, mc * 128 : (mc + 1) * 128],
                                QcTs[lb][:, j, :],
                                start=True,
                                stop=True,
                            )
                            pt = ptpool.tile([128, LB], F32R, tag="pt")
                            nc.scalar.activation(
                                out=pt,
                                in_=s_ps,
                                func=EXP,
                                bias=maskb[:, mc : mc + 1],
                                scale=SCALE,
                            )
                            nc.tensor.matmul(
                                pv,
                                Vc[:, mc, j, :],
                                pt,
                                start=(mc == 0),
                                stop=(mc == NKC - 1),
                            )
                            if mc == 1:
                                nc.gpsimd.tensor_add(acc, pts[0], pt)
                            elif mc > 1:
                                nc.gpsimd.tensor_add(acc, acc, pt)
                            pts.append(pt)
                            if pending is not None:
                                if mc == 1:
                                    tail_a(pending)
                                elif mc == 4:
                                    tail_b(pending)
                                    pending = None

                        csb = accpool.tile([128, LB], F32, tag="csb")
                        nc.gpsimd.partition_all_reduce(
                            csb, acc, 128, bass_isa.ReduceOp.add
                        )
                        if pending is not None:
                            # NKC too small to hit the checkpoints: flush now
                            if "pgt" not in pending:
                                tail_a(pending)
                            tail_b(pending)
                        pending = {"j": j, "pv": pv, "csb": csb}
                    tail_a(pending)
                    tail_b(pending)

                    # Output projection: contract head pairs at K=128.
                    for nt in range(NNT if "3" in phases else 0):
                        nsl = slice(nt * 128, (nt + 1) * 128)
                        por = pmm.tile([128, LB], F32, tag="mm")
                        poi = pmm.tile([128, LB], F32, tag="mm")
                        for p in range(NPAIR):
                            nc.tensor.matmul(
                                por,
                                wo_r_sb[:, p, nsl],
                                outf_r[p],
                                start=(p == 0),
                                stop=(p == NPAIR - 1),
                            )
                            nc.tensor.matmul(
                                poi,
                                wo_i_sb[:, p, nsl],
                                outf_i[p],
                                start=(p == 0),
                                stop=(p == NPAIR - 1),
                            )
                        half = nt % 2
                        if half == 0:
                            ors = ostage.tile(
                                [128, 2, LB], F32, tag="or", name=f"ors{lb}_{nt}"
                            )
                            ois = ostage.tile(
                                [128, 2, LB], F32, tag="oi", name=f"ois{lb}_{nt}"
                            )
                        nc.vector.tensor_copy(out=ors[:, half, :], in_=por)
                        nc.vector.tensor_copy(out=ois[:, half, :], in_=poi)
                        if half == 1:
                            dsl = slice((nt - 1) * 128, (nt + 1) * 128)
                            nc.sync.dma_start(
                                out=out_r[dsl, lsl].rearrange(
                                    "(h p) l -> p h l", p=128
                                ),
                                in_=ors,
                            )
                            nc.sync.dma_start(
                                out=out_i[dsl, lsl].rearrange(
                                    "(h p) l -> p h l", p=128
                                ),
                                in_=ois,
                            )

    nc.finalize()
    return nc


_NC_CACHE = {}


def _get_nc(NKC=9, NKB=5):
    if (NKC, NKB) not in _NC_CACHE:
        _NC_CACHE[(NKC, NKB)] = _build_bass(NKC, NKB)
    return _NC_CACHE[(NKC, NKB)]


def shard_inputs(inputs):
    """Build the 8 per-core input maps; returns (in_maps, LK)."""
    f = lambda k: np.ascontiguousarray(np.asarray(inputs[k], dtype=np.float32))
    r, i = f("r"), f("i")
    mask = np.asarray(inputs["attn_mask"])
    Wqr, Wqi = f("Wqr"), f("Wqi")
    Wkr, Wki = f("Wkr"), f("Wki")
    Wvr, Wvi = f("Wvr"), f("Wvi")
    Wor, Woi = f("Wor"), f("Woi")
    gkr, gki, gvr, gvi = f("gkr"), f("gki"), f("gvr"), f("gvi")
    mix = float(1.0 / (1.0 + np.exp(-np.float32(inputs["gmix"]))))

    # permutation putting unmasked keys first (stable within groups)
    perms = [np.argsort(mask[b], kind="stable") for b in range(B)]
    nks = [int((mask[b] == 0).sum()) for b in range(B)]
    NKC = max(1, (max(nks) + 127) // 128)   # attention key chunks
    NKB = max(1, (max(nks) + PB - 1) // PB)  # K/V projection blocks
    LK = NKC * 128

    in_maps = []
    for core in range(NCORES):
        b, pg = divmod(core, 4)
        heads = range(pg * HPC, (pg + 1) * HPC)
        perm = perms[b]
        nk = nks[b]

        def blocked(x_ld, nblk):  # [seq, D] -> [128, nblk, NDC, PB]
            return np.ascontiguousarray(
                x_ld.reshape(nblk, PB, NDC, 128).transpose(3, 0, 2, 1)
            )

        wq_r = np.empty((D, NPAIR, 128), np.float32)
        wq_i = np.empty((D, NPAIR, 128), np.float32)
        wk_r = np.empty((D, NPAIR, 128), np.float32)
        wk_i = np.empty((D, NPAIR, 128), np.float32)
        wo_r = np.empty((NPAIR, 128, D), np.float32)
        wo_i = np.empty((NPAIR, 128, D), np.float32)
        gkc = np.empty((HPC, 2 * DH, G), np.float32)
        gvc = np.empty((HPC, G, 2 * DH), np.float32)
        for jj, h in enumerate(heads):
            hc = slice(h * DH, (h + 1) * DH)
            p_idx, s_idx = divmod(jj, 2)
            ssl = slice(s_idx * DH, (s_idx + 1) * DH)
            wq_r[:, p_idx, ssl] = Wqr[:, hc]
            wq_i[:, p_idx, ssl] = Wqi[:, hc]
            wk_r[:, p_idx, ssl] = Wkr[:, hc]
            wk_i[:, p_idx, ssl] = Wki[:, hc]
            wo_r[p_idx, ssl, :] = Wor[hc, :]
            wo_i[p_idx, ssl, :] = Woi[hc, :]
            gkc[jj, 0:DH] = gkr[h].T
            gkc[jj, DH:] = gki[h].T
            gvc[jj, :, 0:DH] = gvr[h] * mix
            gvc[jj, :, DH:] = gvi[h] * mix

        cols = slice(pg * CPH, (pg + 1) * CPH)
        bias = np.full(LK, np.float32(MASK_BIAS), np.float32)
        bias[:nk] = 0.0
        in_maps.append(
            {
                "rT": blocked(r[b][perm], NPB),
                "iT": blocked(i[b][perm], NPB),
                "wq_r": wq_r,
                "wq_i": wq_i,
                "wk_r": wk_r,
                "wk_i": wk_i,
                "wv_r": np.ascontiguousarray(Wvr[:, cols]),
                "wv_i": np.ascontiguousarray(Wvi[:, cols]),
                "wo_r": wo_r,
                "wo_i": wo_i,
                "gkc": gkc,
                "gvc": gvc,
                "maskb": np.ascontiguousarray(
                    bias.reshape(LK // 128, 128).T
                ),
            }
        )
    return in_maps, (NKC, NKB), perms


def combine_outputs(results, perms):
    """Sum per-core partials and undo the sequence permutation."""
    out_r = np.zeros((B, L, D), np.float32)
    out_i = np.zeros((B, L, D), np.float32)
    for core, rmap in enumerate(results):
        b = core // 4
        out_r[b, perms[b]] += rmap["out_r"].T
        out_i[b, perms[b]] += rmap["out_i"].T
    return out_r, out_i


def kernel(**inputs):
    in_maps, (NKC, NKB), perms = shard_inputs(inputs)
    nc = _get_nc(NKC, NKB)
    res = run_bass_kernel_spmd(nc, in_maps, core_ids=list(range(NCORES)))
    return combine_outputs(res.results, perms)



# revision 27
# speedup vs baseline: 1.3501x; 1.3501x over previous
"""Trainium2 Bass kernel for complex-valued sparse attention.

Model (B=2, L=2048, D=1024, H=16 heads, DH=64, G=64 global tokens):
  Q/K/V complex projections, real-part scores softmax(Re(Q K^H)) with key
  mask, plus a learned global-token branch, then complex output projection.

Sharding: 8 cores = 2 (batch) x 4 (head groups of 4 heads).  Each core
computes its batch element restricted to its 4 heads end-to-end (column
shards of Wq/Wk/Wv, row shards of Wo) and returns a partial [D, L] output
(transposed, bf16); the host sums the 4 head-group partials per batch.

Key ideas:
  - SPARSITY: masked keys contribute exp(-inf)=0, so the host gathers the
    unmasked key positions (~L/2) and the kernel only projects/attends over
    LK = NKC*128 gathered keys; pad columns get a -60 additive bias.
  - bf16 everywhere on the PE (full rate at any free size), fp32 PSUM.
  - Seq-transposed per-head activations: QcT/KcT are [c=128, seq] where
    c = (64 real | 64 imag); score matmuls contract all 128 partitions.
    Scores are built transposed (keys on partitions) so P@V needs no
    transpose.
  - Attention runs per HEAD PAIR: chunk scores for heads (2h, 2h+1) land in
    one 2-bank PSUM tile and share a single exp (one activation covers
    1024 columns, same per-partition mask bias).
  - Global branch: gk/gv tables are duplicated along G (64 -> 128 rows) so
    the denominator comes from a gpsimd partition_all_reduce instead of a
    PE ones-matmul, and sg/gv matmuls still use all 128 partitions.
  - Softmax skips max-subtraction (scores are O(1) here).  Denominators
    accumulate in fp32 via a binary-counter add tree on the Pool engine
    (SBUF only; Pool cannot touch PSUM on TRN2).
  - PE emission order backfills the ACT-bound attention stream with the
    next l-block's Q projection and the previous l-block's Wo projection,
    so the tensor engine never waits for exp results.
"""

import numpy as np

import concourse.mybir as mybir
import concourse.tile as tile
from concourse import bacc, bass_isa
from concourse.bass_utils import run_bass_kernel_spmd

B, L, D, H, G = 2, 2048, 1024, 16, 64
DH = D // H            # 64 dims per head
HPC = 4                # heads per core
NPAIR = HPC // 2       # head pairs per core
CPH = HPC * DH         # 256 projection columns per core
NCORES = 8
SCALE = DH ** -0.5     # 0.125
PB = 512               # input/projection block width
NPB = L // PB          # 4
LB = 512               # l-block width in attention / output phases
NLB = L // LB          # 4
NDC = D // 128         # 8 contraction chunks of 128
NNT = D // 128         # 8 output-column tiles
MASK_BIAS = -60.0      # additive pre-softmax bias for masked/pad keys

F32 = mybir.dt.float32
BF16 = mybir.dt.bfloat16
EXP = mybir.ActivationFunctionType.Exp


def _build_bass(NKC):
    LK = NKC * 128
    NKB = (LK + PB - 1) // PB
    KBW = [min(PB, LK - PB * b) for b in range(NKB)]  # e.g. [512, 512, 128]

    nc = bacc.Bacc()
    din = lambda name, shape, dt=BF16: nc.dram_tensor(
        name, shape, dt, kind="ExternalInput"
    ).ap()
    rT = din("rT", [128, NPB, NDC, PB])
    iT = din("iT", [128, NPB, NDC, PB])
    wq_r = din("wq_r", [128, NDC, NPAIR, 128])
    wq_i = din("wq_i", [128, NDC, NPAIR, 128])
    wk_r = din("wk_r", [128, NDC, NPAIR, 128])
    wk_i = din("wk_i", [128, NDC, NPAIR, 128])
    wv_r = din("wv_r", [128, NDC, CPH])
    wv_i = din("wv_i", [128, NDC, CPH])
    wo_d = din("wo_ri", [128, 2, NPAIR, D])
    gkc_d = din("gkc", [128, HPC, 128])
    gvc_d = din("gvc", [128, HPC, 128])
    maskb_d = din("maskb", [128, NKC], F32)
    out_ri = nc.dram_tensor(
        "out_ri", [NNT, NLB, 128, 2, LB], BF16, kind="ExternalOutput"
    ).ap()

    with tile.TileContext(nc) as tc:
        with (
            nc.allow_low_precision("bf16 tiles feed full-rate matmuls"),
            tc.tile_pool(name="persist", bufs=1) as persist,
            tc.tile_pool(name="inpool", bufs=3) as inpool,
            tc.tile_pool(name="ptpool", bufs=6) as ptpool,
            tc.tile_pool(name="pgpool", bufs=2) as pgpool,
            tc.tile_pool(name="denpool", bufs=3) as denpool,
            tc.tile_pool(name="outfpool", bufs=2) as outfpool,
            tc.tile_pool(name="ostpool", bufs=3) as ostpool,
            tc.tile_pool(name="sp", bufs=3, space="PSUM") as sp,
            tc.tile_pool(name="pvp", bufs=2, space="PSUM") as pvp,
        ):
            # ---------------- persistent SBUF tiles ----------------
            wsb = {}
            for name in ("wq_r", "wq_i", "wk_r", "wk_i"):
                wsb[name] = persist.tile(
                    [128, NDC, NPAIR, 128], BF16, tag=name, name=name
                )
            wv_r_sb = persist.tile([128, NDC, CPH], BF16, tag="wvr")
            wv_i_sb = persist.tile([128, NDC, CPH], BF16, tag="wvi")
            wo_sb = persist.tile([128, 2, NPAIR, D], BF16, tag="wo")
            KcT = persist.tile([128, HPC, LK], BF16, tag="kc")
            Vc = persist.tile([128, NKC, HPC, 128], BF16, tag="vc")
            QcTs = [
                persist.tile([128, HPC, LB], BF16, tag=f"qc{t}", name=f"QcT{t}")
                for t in range(NLB)
            ]
            gkc = persist.tile([128, HPC, 128], BF16, tag="gkc")
            gvc = persist.tile([128, HPC, 128], BF16, tag="gvc")
            maskb = persist.tile([128, NKC], F32, tag="mask")

            # prologue DMAs (SP queue), ordered so the first matmuls of
            # Q-lb0 / K-b0 have their operands as early as possible
            in_tiles = {}

            def load_input(pb):
                rt = inpool.tile([128, NDC, PB], BF16, tag="rt", name=f"rt{pb}")
                it = inpool.tile([128, NDC, PB], BF16, tag="it", name=f"it{pb}")
                nc.sync.dma_start(out=rt, in_=rT[:, pb])
                nc.sync.dma_start(out=it, in_=iT[:, pb])
                in_tiles[pb] = (rt, it)

            # prologue DMAs fan out across the SP / ACT / Pool queues so the
            # first projection matmuls start as early as possible
            rt0 = inpool.tile([128, NDC, PB], BF16, tag="rt", name="rt0")
            it0 = inpool.tile([128, NDC, PB], BF16, tag="it", name="it0")
            in_tiles[0] = (rt0, it0)
            nc.sync.dma_start(out=wsb["wq_r"][:, :, 0], in_=wq_r[:, :, 0])
            nc.scalar.dma_start(out=rt0[:, 0:4], in_=rT[:, 0, 0:4])
            nc.gpsimd.dma_start(out=it0[:, 0:4], in_=iT[:, 0, 0:4])
            nc.sync.dma_start(out=wsb["wq_r"][:, :, 1], in_=wq_r[:, :, 1])
            nc.sync.dma_start(out=wsb["wq_i"], in_=wq_i)
            nc.scalar.dma_start(out=rt0[:, 4:8], in_=rT[:, 0, 4:8])
            nc.gpsimd.dma_start(out=it0[:, 4:8], in_=iT[:, 0, 4:8])
            nc.sync.dma_start(out=wsb["wk_r"], in_=wk_r)
            nc.sync.dma_start(out=wsb["wk_i"], in_=wk_i)
            nc.gpsimd.dma_start(out=maskb, in_=maskb_d)
            nc.sync.dma_start(out=wv_r_sb, in_=wv_r)
            nc.sync.dma_start(out=wv_i_sb, in_=wv_i)
            load_input(1)
            nc.sync.dma_start(out=gkc, in_=gkc_d)
            nc.sync.dma_start(out=gvc, in_=gvc_d)
            load_input(2)
            nc.sync.dma_start(out=wo_sb, in_=wo_d)
            load_input(3)

            # weighted round-robin PSUM->SBUF copies: ACT carries the exp
            # stream, so DVE takes 2 of every 3 copies
            cp_state = [0]

            def copy_rr(out, in_):
                if cp_state[0] % 3 == 0:
                    nc.scalar.copy(out=out, in_=in_)
                else:
                    nc.vector.tensor_copy(out=out, in_=in_)
                cp_state[0] += 1

            # ------------- projection helpers (all bf16) -------------
            def proj_qk(w_r, w_i, rt, it, src_off, w, dst, dst_off):
                """One head-pair r+i projection block into a 2-bank slot.

                Computes [pair cols x w] for the r and i systems, then
                repacks into per-head (r|i) c-layout at dst[:, head] cols
                [dst_off : dst_off+w].
                """
                for p in range(NPAIR):
                    slot = sp.tile([128, 2, LB], F32, tag="sp")
                    for c in range(NDC):
                        nc.tensor.matmul(
                            slot[:, 0, :w],
                            w_r[:, c, p, :],
                            rt[:, c, src_off : src_off + w],
                            start=(c == 0),
                            stop=(c == NDC - 1),
                        )
                    for c in range(NDC):
                        nc.tensor.matmul(
                            slot[:, 1, :w],
                            w_i[:, c, p, :],
                            it[:, c, src_off : src_off + w],
                            start=(c == 0),
                            stop=(c == NDC - 1),
                        )
                    dsl = slice(dst_off, dst_off + w)
                    copy_rr(dst[0:DH, 2 * p, dsl], slot[0:DH, 0, :w])
                    copy_rr(dst[0:DH, 2 * p + 1, dsl], slot[DH:128, 0, :w])
                    copy_rr(dst[DH:128, 2 * p, dsl], slot[0:DH, 1, :w])
                    copy_rr(dst[DH:128, 2 * p + 1, dsl], slot[DH:128, 1, :w])

            def proj_v(mc):
                """V projection for key chunk mc -> Vc[:, mc] (bf16)."""
                pb, off = divmod(mc * 128, PB)
                rt, it = in_tiles[pb]
                slot = sp.tile([128, 2, LB], F32, tag="sp")
                for c in range(NDC):
                    nc.tensor.matmul(
                        slot[:, 0, 0:CPH],
                        rt[:, c, off : off + 128],
                        wv_r_sb[:, c, :],
                        start=(c == 0),
                        stop=(c == NDC - 1),
                    )
                for c in range(NDC):
                    nc.tensor.matmul(
                        slot[:, 1, 0:CPH],
                        it[:, c, off : off + 128],
                        wv_i_sb[:, c, :],
                        start=(c == 0),
                        stop=(c == NDC - 1),
                    )
                copy_rr(
                    Vc[:, mc, :, 0:DH],
                    slot[:, 0, 0:CPH].rearrange("p (j d) -> p j d", d=DH),
                )
                copy_rr(
                    Vc[:, mc, :, DH:128],
                    slot[:, 1, 0:CPH].rearrange("p (j d) -> p j d", d=DH),
                )

            def proj_q(lb):
                rt, it = in_tiles[lb]
                proj_qk(
                    wsb["wq_r"], wsb["wq_i"], rt, it, 0, PB, QcTs[lb], 0
                )

            def proj_q_fillers(lb):
                """Q projection for one l-block, split per head pair."""
                rt, it = in_tiles[lb]

                def one(p):
                    slot = sp.tile([128, 2, LB], F32, tag="sp")
                    for ri, (w_sb, src) in enumerate(
                        ((wsb["wq_r"], rt), (wsb["wq_i"], it))
                    ):
                        for c in range(NDC):
                            nc.tensor.matmul(
                                slot[:, ri, :],
                                w_sb[:, c, p, :],
                                src[:, c, :],
                                start=(c == 0),
                                stop=(c == NDC - 1),
                            )
                    copy_rr(QcTs[lb][0:DH, 2 * p, :], slot[0:DH, 0, :])
                    copy_rr(QcTs[lb][0:DH, 2 * p + 1, :], slot[DH:128, 0, :])
                    copy_rr(QcTs[lb][DH:128, 2 * p, :], slot[0:DH, 1, :])
                    copy_rr(QcTs[lb][DH:128, 2 * p + 1, :], slot[DH:128, 1, :])

                return [lambda p=p: one(p) for p in range(NPAIR)]

            # ---------------- attention per head pair ----------------
            def emit_wo(lb, nts, fast=()):
                """Wo projection for l-block lb over output tiles nts.

                Tiles in `fast` split their drain across two engines and two
                DMA queues to shorten the end-of-program tail.
                """
                for nt in nts:
                    nsl = slice(nt * 128, (nt + 1) * 128)
                    slot = sp.tile([128, 2, LB], F32, tag="sp")
                    for ri in range(2):
                        for p in range(NPAIR):
                            nc.tensor.matmul(
                                slot[:, ri, :],
                                wo_sb[:, ri, p, nsl],
                                outf[lb % 2][ri][p],
                                start=(p == 0),
                                stop=(p == NPAIR - 1),
                            )
                    ost = ostpool.tile([128, 2, LB], BF16, tag="ost")
                    # r/i copies run on ACT and DVE in parallel: halves the
                    # slot-free latency and spreads the copy load
                    nc.scalar.copy(out=ost[:, 0, :], in_=slot[:, 0, :])
                    nc.vector.tensor_copy(out=ost[:, 1, :], in_=slot[:, 1, :])
                    if nt in fast:
                        nc.sync.dma_start(
                            out=out_ri[nt, lb, :, 0, :], in_=ost[:, 0, :]
                        )
                        nc.gpsimd.dma_start(
                            out=out_ri[nt, lb, :, 1, :], in_=ost[:, 1, :]
                        )
                    else:
                        nc.sync.dma_start(out=out_ri[nt, lb], in_=ost)

            outf = [None, None]  # rotating per-lb outf tiles

            def attn_pair(lb, hp, filler, pop_from=0):
                """Scores+softmax+PV+global+tails for heads (2hp, 2hp+1).

                `filler` is a list of zero-arg callables emitting backfill
                PE work (projections / Wo); they are spread through the
                chunk loop so the PE stream stays ahead of ACT.
                """
                j0, j1 = 2 * hp, 2 * hp + 1
                pv = {
                    jj: pvp.tile([128, LB], F32, tag="pv", name=f"pv{lb}_{hp}_{jj}")
                    for jj in (0, 1)
                }
                # binary-counter fp32 denominator accumulation (Pool)
                counters = {0: {}, 1: {}}
                acc_n = [0]

                def push_den(jj, x):
                    level, cnt = 0, counters[jj]
                    while level in cnt:
                        prev = cnt.pop(level)
                        t = denpool.tile(
                            [128, LB], F32, tag=f"acc{level}",
                            name=f"acc{lb}_{hp}_{jj}_{acc_n[0]}",
                        )
                        acc_n[0] += 1
                        nc.gpsimd.tensor_add(t, prev, x)
                        x = t
                        level += 1
                    cnt[level] = x

                def emit_pv(mc):
                    for jj, j in ((0, j0), (1, j1)):
                        nc.tensor.matmul(
                            pv[jj], Vc[:, mc, j, :], pts[mc][:, jj, :],
                            start=(mc == 0), stop=(mc == NKC - 1),
                        )
                        push_den(jj, pts[mc][:, jj, :])

                # global-branch scores first: gives ACT early work and gets
                # the global denominator off the tail critical path
                gslot = sp.tile([128, 2, LB], F32, tag="sp")
                nc.tensor.matmul(
                    gslot[:, 0, :], gkc[:, j0, :], QcTs[lb][:, j0, :],
                    start=True, stop=True,
                )
                nc.tensor.matmul(
                    gslot[:, 1, :], gkc[:, j1, :], QcTs[lb][:, j1, :],
                    start=True, stop=True,
                )
                pgt = pgpool.tile([128, 2, LB], BF16, tag="pg")
                nc.scalar.activation(
                    out=pgt, in_=gslot, func=EXP, bias=0.0, scale=SCALE
                )

                pts = []
                for mc in range(NKC):
                    if filler and mc >= pop_from:
                        filler.pop(0)()
                    slot = sp.tile([128, 2, LB], F32, tag="sp")
                    msl = slice(mc * 128, (mc + 1) * 128)
                    nc.tensor.matmul(
                        slot[:, 0, :], KcT[:, j0, msl], QcTs[lb][:, j0, :],
                        start=True, stop=True,
                    )
                    nc.tensor.matmul(
                        slot[:, 1, :], KcT[:, j1, msl], QcTs[lb][:, j1, :],
                        start=True, stop=True,
                    )
                    pt = ptpool.tile([128, 2, LB], BF16, tag="pt")
                    nc.scalar.activation(
                        out=pt, in_=slot, func=EXP,
                        bias=maskb[:, mc : mc + 1], scale=SCALE,
                    )
                    pts.append(pt)
                    if mc > 1:
                        emit_pv(mc - 2)  # PV lags scores by two chunks so the
                        # in-order PE queue never waits for the exp result
                emit_pv(NKC - 2)

                # global values (G duplicated to 128 rows); emitted between
                # the last two PV chunks to cover the final exp latency
                gvslot = sp.tile([128, 2, LB], F32, tag="sp")
                for jj, j in ((0, j0), (1, j1)):
                    nc.tensor.matmul(
                        gvslot[:, jj, :], gvc[:, j, :], pgt[:, jj, :],
                        start=True, stop=True,
                    )
                emit_pv(NKC - 1)
                while filler:
                    filler.pop(0)()

                # tails: finish denominators, normalize, scatter into outf
                for jj, j in ((0, j0), (1, j1)):
                    cnt = counters[jj]
                    x = None
                    for level in sorted(cnt):
                        if x is None:
                            x = cnt[level]
                        else:
                            t = denpool.tile(
                                [128, LB], F32, tag=f"fin{level}",
                                name=f"fin{lb}_{hp}_{jj}_{level}",
                            )
                            nc.gpsimd.tensor_add(t, x, cnt[level])
                            x = t
                    den_b = denpool.tile(
                        [128, LB], F32, tag="dnb", name=f"dnb{lb}_{hp}_{jj}"
                    )
                    nc.gpsimd.partition_all_reduce(
                        den_b, x, 128, bass_isa.ReduceOp.add
                    )
                    gdn_b = denpool.tile(
                        [128, LB], F32, tag="gdb", name=f"gdb{lb}_{hp}_{jj}"
                    )
                    nc.gpsimd.partition_all_reduce(
                        gdn_b, pgt[:, jj, :], 128, bass_isa.ReduceOp.add
                    )
                    rcb = denpool.tile(
                        [128, LB], F32, tag="rcb", name=f"rcb{lb}_{hp}_{jj}"
                    )
                    nc.vector.reciprocal(rcb, den_b)
                    rcg = denpool.tile(
                        [128, LB], F32, tag="rcg", name=f"rcg{lb}_{hp}_{jj}"
                    )
                    nc.vector.reciprocal(rcg, gdn_b)
                    t1 = denpool.tile(
                        [128, LB], BF16, tag="t1", name=f"t1{lb}_{hp}_{jj}"
                    )
                    nc.vector.tensor_mul(t1, pv[jj], rcb)
                    t2 = denpool.tile(
                        [128, LB], BF16, tag="t2", name=f"t2{lb}_{hp}_{jj}"
                    )
                    nc.vector.tensor_mul(t2, gvslot[:, jj, :], rcg)
                    hsl = slice(jj * DH, (jj + 1) * DH)
                    nc.gpsimd.tensor_add(
                        outf[lb % 2][0][hp][hsl, :], t1[0:DH, :], t2[0:DH, :]
                    )
                    nc.gpsimd.tensor_add(
                        outf[lb % 2][1][hp][hsl, :], t1[DH:128, :], t2[DH:128, :]
                    )

            def fresh_outf(lb):
                outf[lb % 2] = [
                    [
                        outfpool.tile(
                            [128, LB], BF16, tag=f"of{ri}{p}",
                            name=f"outf{lb}_{ri}_{p}",
                        )
                        for p in range(NPAIR)
                    ]
                    for ri in range(2)
                ]

            # ================== emission schedule ==================
            # Phase A: Q-lb0, K, then V chunks interleaved with the first
            # head pair's attention so ACT starts early.
            proj_q(0)
            koff = 0
            for kb, w in enumerate(KBW):
                pb = koff // PB
                rt, it = in_tiles[pb]
                proj_qk(
                    wsb["wk_r"], wsb["wk_i"], rt, it, koff - pb * PB, w,
                    KcT, koff,
                )
                koff += w

            fresh_outf(0)
            proj_v(0)
            filler = [lambda mc=mc: proj_v(mc) for mc in range(1, NKC)]
            attn_pair(0, 0, filler)

            # Phase B: remaining head pairs / l-blocks with backfill
            for lb in range(NLB):
                for hp in range(NPAIR):
                    if lb == 0 and hp == 0:
                        continue  # done in phase A
                    filler = []
                    if hp == 0:
                        fresh_outf(lb)
                    if lb < NLB - 1 and hp == 1:
                        # next l-block Q projection (2 pair-groups) first:
                        # it has no dependency on the previous pair's tails
                        filler += proj_q_fillers(lb + 1)
                    if lb > 0:
                        # previous l-block Wo projection, 4 tiles per pair
                        filler += [
                            lambda l=lb - 1, n=nt: emit_wo(l, (n,))
                            for nt in range(4 * hp, 4 * hp + 4)
                        ]
                    # hp0's Wo fillers need the previous pair's tail chain to
                    # finish, so they pop a little later
                    attn_pair(lb, hp, filler, pop_from=3 if hp == 0 else 2)

            emit_wo(NLB - 1, range(NNT), fast=(NNT - 2, NNT - 1))

    nc.finalize()
    return nc


_NC_CACHE = {}


def _get_nc(NKC=9):
    if NKC not in _NC_CACHE:
        _NC_CACHE[NKC] = _build_bass(NKC)
    return _NC_CACHE[NKC]


def _bf16(x):
    import ml_dtypes

    return np.ascontiguousarray(np.asarray(x, np.float32)).astype(
        ml_dtypes.bfloat16
    )


def shard_inputs(inputs):
    """Build the 8 per-core input maps; returns (in_maps, NKC, perms)."""
    f = lambda k: np.asarray(inputs[k], dtype=np.float32)
    r, i = f("r"), f("i")
    mask = np.asarray(inputs["attn_mask"])
    Wq = (f("Wqr"), f("Wqi"))
    Wk = (f("Wkr"), f("Wki"))
    Wv = (f("Wvr"), f("Wvi"))
    Wo = (f("Wor"), f("Woi"))
    gk = (f("gkr"), f("gki"))
    gv = (f("gvr"), f("gvi"))
    mix = float(1.0 / (1.0 + np.exp(-np.float32(inputs["gmix"]))))

    perms = [np.argsort(mask[b], kind="stable") for b in range(B)]
    nks = [int((mask[b] == 0).sum()) for b in range(B)]
    NKC = max(1, (max(nks) + 127) // 128)
    LK = NKC * 128

    def blocked(x_ld):  # [seq, D] -> [128, NPB, NDC, PB]
        return _bf16(
            x_ld.reshape(NPB, PB, NDC, 128).transpose(3, 0, 2, 1)
        )

    def wqk(Wm, cols):  # [D, 256] -> [128, NDC, NPAIR, 128]
        return _bf16(
            Wm[:, cols].reshape(NDC, 128, NPAIR, 128).transpose(1, 0, 2, 3)
        )

    in_maps = []
    for core in range(NCORES):
        b, pg = divmod(core, 4)
        heads = range(pg * HPC, (pg + 1) * HPC)
        perm = perms[b]
        nk = nks[b]
        cols = slice(pg * CPH, (pg + 1) * CPH)

        wo = np.empty((128, 2, NPAIR, D), np.float32)
        gkc = np.empty((128, HPC, 128), np.float32)
        gvc = np.empty((128, HPC, 128), np.float32)
        for jj, h in enumerate(heads):
            hc = slice(h * DH, (h + 1) * DH)
            p_idx, s_idx = divmod(jj, 2)
            ssl = slice(s_idx * DH, (s_idx + 1) * DH)
            wo[ssl, 0, p_idx] = Wo[0][hc, :]
            wo[ssl, 1, p_idx] = Wo[1][hc, :]
            gkc[0:DH, jj] = np.concatenate([gk[0][h].T, gk[0][h].T], axis=1)
            gkc[DH:128, jj] = np.concatenate([gk[1][h].T, gk[1][h].T], axis=1)
            gvh = np.concatenate([gv[0][h], gv[1][h]], axis=1) * mix
            gvc[:, jj] = np.concatenate([gvh, gvh], axis=0)

        bias = np.full(LK, np.float32(MASK_BIAS), np.float32)
        bias[:nk] = 0.0
        in_maps.append(
            {
                "rT": blocked(r[b][perm]),
                "iT": blocked(i[b][perm]),
                "wq_r": wqk(Wq[0], cols),
                "wq_i": wqk(Wq[1], cols),
                "wk_r": wqk(Wk[0], cols),
                "wk_i": wqk(Wk[1], cols),
                "wv_r": _bf16(
                    Wv[0][:, cols].reshape(NDC, 128, CPH).transpose(1, 0, 2)
                ),
                "wv_i": _bf16(
                    Wv[1][:, cols].reshape(NDC, 128, CPH).transpose(1, 0, 2)
                ),
                "wo_ri": _bf16(wo),
                "gkc": _bf16(gkc),
                "gvc": _bf16(gvc),
                "maskb": np.ascontiguousarray(bias.reshape(NKC, 128).T),
            }
        )
    return in_maps, NKC, perms


def core_partial(out_ri):
    """Decode one core's out_ri [NNT, NLB, 128, 2, LB] -> (r, i) [D, L]."""
    o = np.asarray(out_ri, dtype=np.float32)
    out_r = o[:, :, :, 0, :].transpose(0, 2, 1, 3).reshape(D, L)
    out_i = o[:, :, :, 1, :].transpose(0, 2, 1, 3).reshape(D, L)
    return out_r, out_i


def combine_outputs(results, perms):
    """Sum per-core partials and undo the sequence permutation."""
    out_r = np.zeros((B, L, D), np.float32)
    out_i = np.zeros((B, L, D), np.float32)
    for core, rmap in enumerate(results):
        b = core // 4
        pr, pi = core_partial(rmap["out_ri"])
        out_r[b, perms[b]] += pr.T
        out_i[b, perms[b]] += pi.T
    return out_r, out_i


def kernel(**inputs):
    in_maps, NKC, perms = shard_inputs(inputs)
    nc = _get_nc(NKC)
    res = run_bass_kernel_spmd(nc, in_maps, core_ids=list(range(NCORES)))
    return combine_outputs(res.results, perms)
